# revision 7
# baseline (speedup 1.0000x reference)
"""Trainium2 Bass kernel for nn_DeformConv2d_3246995276085 — v2.

Same structural insight as v1 (only slab q=0 / an 11x11 pixel corner
contributes; everything else exactly zero), with a restructured
dataflow:

1. d-major gather element order j = 128*d + pix: element j lands at VV
   partition j%128 = pix, chunk j//128 = d.  The wrapped-16 int16 index
   tile the DGE needs (idx j at partition j%16, column j//16) is
   IDXC[p, 8d+q] = slot[16q + p%16, d] — built ON-CHIP with one bias
   matmul (+99) and 8 tiny fold matmuls (lhsT = block one-hots at base
   partitions 0/32/64/96).  Kills v1's DRAM-roundtrip index pipeline.

2. Pair-slot image: host stages xhp[R,C] = the 2x2 pixel patch with
   top-left (R-1, C-1) as 256 bf16 = 512B per slot; ONE gather fetches
   all 4 bilinear corners.  Clip-validity of the +1 row/col is restored
   by two weight factors (ax1 *= IX0>=-1, wy1 *= IY0>=-1).

3. The offset conv is fp32 (coordinate accuracy: offsets are in
   normalized units x48!) but emitted TRANSPOSED — lhsT = the x window,
   rhs = host-preblended weights 48*(a*w1+(1-a)*w2) with dx-pairs
   stacked on 128 partitions — so psum comes out [66pix, 18] in OCT
   layout directly, free size 18.  The x48+47.5 affine and the alpha
   blend live in host-prepared weights/bases.

4. Per-(pix,d) weights stay on-chip [128, 9]; bilinear combine is
   5-op per-d chains (4 corners + modulation) split DVE (d0-5) /
   gpsimd (d6-8); modulation is reshaped run->[pix,d] by one DMA.

Mod conv and everything else on PE runs bf16 with dx-tap pairs.
"""

import functools
import os

import numpy as np

KDEBUG = bool(os.environ.get("KDEBUG"))

ND = 9
C = 64
H = W = 96
NJ = 11
NSTRIP = 6
NPIX = 128       # partition domain (66 real)
XHROWS = 98 * 98  # pair-slot image rows
DUMMY = 1.0e5

DIRY = np.array([0, 0, 0, 1, 1, 1, -1, -1, -1], np.float32)
DIRX = np.array([0, 1, -1, 0, 1, -1, 0, 1, -1], np.float32)

# blobW bf16 [128 rows]: WCNV 0:576, IDENT16 576:704
BW_WCNV = 0
BW_IDENT16 = 576
BW_COLS = 704

# blobB fp32: blobb1 = offset conv + base; blobb2 = wrapped-16 fold consts
BB_XW3 = 0        # [128, 3, 66] pair windows (dx=-1 half-1 / dx=0 half-2)
BB_XW3S = 198     # [128, 3, 66] single windows (dx=+1, rows 0:64) + ones row
BB_WOFFB2 = 396   # [128, 3, 18] 48*blended w pairs
BB_WOFF1S = 450   # [128, 3, 18] 48*blended w singles + bias row at p=64
BB_BGI = 504      # [128, 18]    48*(base grid) + 47.0  (DUMMY on pads)
BB_BMOD = 522     # [1, 1]
BB_Z18 = 523      # [1, 18] zeros
BB_ONES = 541     # [1, 128]     ones row
BB1_COLS = 669
BB_REPLAB = 669   # [128, 256]
BB_C99 = 925      # [1, 72]      99.0 row
BB_COLS = 997


# ----------------------------------------------------------------- host prep

def _make_xhp(xb, bf16):
    """xb (64,96,96) fp32 -> (9604, 256) bf16 pair-slot image."""
    xp = np.zeros((99, 99, C), np.float32)
    xp[1:97, 1:97] = xb.transpose(1, 2, 0)
    out = np.concatenate([xp[0:98, 0:98], xp[0:98, 1:99],
                          xp[1:99, 0:98], xp[1:99, 1:99]], axis=-1)
    return np.ascontiguousarray(out.reshape(XHROWS, 4 * C)).astype(bf16)


def _make_core_inputs(x, w_off1, b_off1, w_off2, b_off2, w_mod, b_mod,
                      conv_weight, alpha, b, part, xhp):
    import ml_dtypes
    bf16 = ml_dtypes.bfloat16
    i0 = 6 * part
    xb = x[b]

    blobw = np.zeros((128, BW_COLS), np.float32)
    blobw[:, BW_IDENT16:BW_IDENT16 + 128] = np.eye(128, dtype=np.float32)
    wcnv = np.zeros((C, ND, 64), np.float32)
    for t in range(9):
        dy, dx = t // 3, t % 3
        wcnv[:, t, :] = conv_weight[:, :, dy, dx].T
    blobw[0:64, BW_WCNV:BW_WCNV + 576] = wcnv.reshape(C, 576)
    wbl = np.asarray((np.float64(alpha) * w_off1.astype(np.float64)
                      + (1.0 - np.float64(alpha)) * w_off2.astype(np.float64))
                     * 48.0, np.float32)
    bbl = np.asarray((np.float64(alpha) * b_off1.astype(np.float64)
                      + (1.0 - np.float64(alpha)) * b_off2.astype(np.float64))
                     * 48.0, np.float32)

    blobb = np.zeros((128, BB_COLS), np.float32)
    xw2 = np.zeros((128, 8, 13), np.float32)
    for r in range(8):
        xr = i0 - 1 + r
        if 0 <= xr < H:
            xw2[0:64, r, 1:12] = xb[:, xr, 0:NJ]
            xw2[64:128, r, 0:12] = xb[:, xr, 0:12]
    xw3 = np.zeros((128, 3, 66), np.float32)
    xw3s = np.zeros((128, 3, 66), np.float32)
    for dyi in range(3):
        # window rows a=0..5 -> xw2 row dyi+a; pairs read cols 0:11,
        # singles (dx=+1) read half-1 cols 2:13
        xw3[:, dyi, :] = xw2[:, dyi:dyi + 6, 0:11].reshape(128, 66)
        xw3s[0:64, dyi, :] = xw2[0:64, dyi:dyi + 6, 2:13].reshape(64, 66)
    xw3s[64, 0, :] = 1.0   # bias carrier row
    blobb[:, BB_XW3:BB_XW3 + 198] = xw3.reshape(128, 198)
    blobb[:, BB_XW3S:BB_XW3S + 198] = xw3s.reshape(128, 198)
    woffb2 = np.zeros((128, 3, 18), np.float32)
    for dyi in range(3):
        woffb2[0:64, dyi, :] = wbl[:, :, dyi, 0].T  # dx=-1
        woffb2[64:128, dyi, :] = wbl[:, :, dyi, 1].T  # dx=0
    blobb[:, BB_WOFFB2:BB_WOFFB2 + 54] = woffb2.reshape(128, 54)
    woff1s = np.zeros((128, 3, 18), np.float32)
    for dyi in range(3):
        woff1s[0:64, dyi, :] = wbl[:, :, dyi, 2].T  # dx=+1
    woff1s[64, 0, :] = bbl   # bias via the ones row
    blobb[:, BB_WOFF1S:BB_WOFF1S + 54] = woff1s.reshape(128, 54)
    blobb[0, BB_ONES:BB_ONES + 128] = 1.0
    blobb[0, BB_C99:BB_C99 + 72] = 99.0
    p = np.arange(128)[:, None]
    cc = np.arange(128)[None, :]
    replab = np.zeros((128, 256), np.float32)
    replab[:, 0:128] = (p % 32 < 16) & (cc % 16 == p % 32)
    replab[:, 128:256] = (p % 32 >= 16) & (cc % 16 == p % 32 - 16)
    blobb[:, BB_REPLAB:BB_REPLAB + 256] = replab
    bgi = np.full((NPIX, 18), DUMMY * 48.0 + 47.0, np.float32)
    for pp in range(NSTRIP * NJ):
        ii, jj = i0 + pp // NJ, pp % NJ
        bgi[pp, 0:9] = (ii + DIRY) * 48.0 + 47.0
        bgi[pp, 9:18] = (jj + DIRX) * 48.0 + 47.0
    blobb[:, BB_BGI:BB_BGI + 18] = bgi
    blobb[0, BB_BMOD] = np.float32(b_mod[0])

    blob128 = np.zeros((128, 6), np.float32)
    blob128[0:64, 0:3] = w_mod[0, :, :, 0]   # dx=-1 per dy
    blob128[64:128, 0:3] = w_mod[0, :, :, 1]  # dx=0
    blob128[0:64, 3:6] = w_mod[0, :, :, 2]   # dx=+1 singles

    xm2 = np.zeros((128, NSTRIP, 4, 98), np.float32)
    for s in range(NSTRIP):
        for r in range(4):
            xr = 9 * (i0 + s) - 1 + r
            if 0 <= xr < H:
                xm2[0:64, s, r, 1:97] = xb[:, xr, :]
                xm2[64:128, s, r, 0:96] = xb[:, xr, :]

    return {
        "xhp": xhp,
        "blobw": blobw.astype(bf16),
        "blobb1": np.ascontiguousarray(blobb[:, 0:BB1_COLS]),
        "blobb2": np.ascontiguousarray(blobb[:, BB1_COLS:]),
        "blob128": blob128.astype(bf16),
        "xm2a": np.ascontiguousarray(
            xm2[:, 0:3].reshape(128, 3 * 4 * 98)).astype(bf16),
        "xm2b": np.ascontiguousarray(
            xm2[:, 3:6].reshape(128, 3 * 4 * 98)).astype(bf16),
    }


# ------------------------------------------------------------- device kernel

def emit_kernel(tc, outs, ins):
    from contextlib import ExitStack

    import concourse.bass as bass
    from concourse import mybir

    ctx = ExitStack()

    dt = mybir.dt
    Alu = mybir.AluOpType
    Act = mybir.ActivationFunctionType
    nc = tc.nc
    f32 = dt.float32
    bf = dt.bfloat16

    strips_out = outs["strips_out"]

    consts = ctx.enter_context(tc.tile_pool(name="consts", bufs=1))
    work = ctx.enter_context(tc.tile_pool(name="work", bufs=1))
    loop_sb = ctx.enter_context(tc.tile_pool(name="loop_sb", bufs=6))
    psA = ctx.enter_context(tc.tile_pool(name="psA", bufs=1, space="PSUM"))
    psB = ctx.enter_context(tc.tile_pool(name="psB", bufs=2, space="PSUM"))
    psC = ctx.enter_context(tc.tile_pool(name="psC", bufs=3, space="PSUM"))
    psD = ctx.enter_context(tc.tile_pool(name="psD", bufs=2, space="PSUM"))

    def ap(t, offset_extra, dims):
        base = t[:] if not isinstance(t, bass.AP) else t
        return bass.AP(tensor=base.tensor, offset=base.offset + offset_extra,
                       ap=dims)

    # ---- input loads
    BLOBB = consts.tile([128, BB_COLS], f32)
    BLOBW = consts.tile([128, BW_COLS], bf)
    nc.sync.dma_start(out=BLOBW, in_=ins["blobw"])
    nc.sync.dma_start(out=BLOBB[:, 0:BB1_COLS], in_=ins["blobb1"])
    nc.sync.dma_start(out=BLOBB[:, BB1_COLS:BB_COLS], in_=ins["blobb2"])
    BLOB128 = consts.tile([128, 6], bf)
    nc.scalar.dma_start(out=BLOB128, in_=ins["blob128"])
    XM2 = consts.tile([128, NSTRIP, 4, 98], bf)
    XM2a = ap(XM2, 0, [XM2[:].ap[0], [1, 3 * 4 * 98]])
    XM2b = ap(XM2, 3 * 4 * 98, [XM2[:].ap[0], [1, 3 * 4 * 98]])
    nc.scalar.dma_start(out=XM2a, in_=ins["xm2a"])
    nc.scalar.dma_start(out=XM2b, in_=ins["xm2b"])

    XW3 = BLOBB[:, BB_XW3:BB_XW3 + 198]
    XW3S = BLOBB[:, BB_XW3S:BB_XW3S + 198]
    WOFFB2 = BLOBB[:, BB_WOFFB2:BB_WOFFB2 + 54]
    WOFF1S = BLOBB[:, BB_WOFF1S:BB_WOFF1S + 54]
    ONES = BLOBB[0:1, BB_ONES:BB_ONES + 128]
    C99 = BLOBB[0:1, BB_C99:BB_C99 + 72]
    REPLAB = BLOBB[:, BB_REPLAB:BB_REPLAB + 256]
    BGI = BLOBB[:, BB_BGI:BB_BGI + 18]
    BMOD = BLOBB[0:1, BB_BMOD:BB_BMOD + 1]
    Z18 = BLOBB[0:1, BB_Z18:BB_Z18 + 18]
    WCNV = BLOBW[0:64, BW_WCNV:BW_WCNV + 576].rearrange("p (a b) -> p a b",
                                                        a=9)
    IDENT16 = BLOBW[:, BW_IDENT16:BW_IDENT16 + 128]
    WMOD2 = BLOB128[:, 0:3]
    WMOD1 = BLOB128[0:64, 3:6]

    # ---- zero-init tiles (gpsimd)
    FP = work.tile([C, NSTRIP, 2, 98], bf)
    nc.gpsimd.memset(FP, 0.0)
    MODT = work.tile([128, ND], f32)
    nc.gpsimd.memset(MODT, 0.0)
    ZB = consts.tile([C, 4, 96], bf)
    nc.gpsimd.memset(ZB, 0.0)

    _after_folds = []
    _after_off = []

    # ---- transposed offset conv (fp32): ps_o[pix, oc] = 48*offset + 48*bias
    # rows of the window: pixel strip ii uses xw2 row 1+ii+dy (xw2 row r
    # holds x row i0-1+r), so for out strips 0..5: rows (1+dy)..(6+dy).
    ps_o = psA.tile([66, 18], f32, tag="psA")
    for i in range(3):
        nc.tensor.matmul(
            ps_o,
            lhsT=XW3[:, 66 * i:66 * i + 66],
            rhs=WOFFB2[:, 18 * i:18 * i + 18],
            start=(i == 0),
            stop=False,
            skip_group_check=True,
        )
    for i in range(3):
        _om = nc.tensor.matmul(
            ps_o,
            lhsT=XW3S[:, 66 * i:66 * i + 66],
            rhs=WOFF1S[:, 18 * i:18 * i + 18],
            start=False,
            stop=(i == 2),
            skip_group_check=True,
        )
    _after_off.append(_om.ins.name)

    # ---- mod conv phi0 strips 0-2 (PE; overlaps DVE idx chain below)
    MODV = work.tile([1, NSTRIP, 99], f32)

    def mod_rows(c2, after_folds=False):
        ps_m = psB.tile([1, 3, 96], f32, tag="ps_m")
        for i, dy in enumerate((-1, 0, 1)):
            mm = nc.tensor.matmul(
                ps_m,
                lhsT=WMOD2[:, 1 + dy:2 + dy],
                rhs=XM2[:, 3 * c2:3 * c2 + 3, 1 + dy:2 + dy, 0:96],
                start=(i == 0), stop=False, skip_group_check=True)
            if i == 0:
                from bass_rust import InstructionNameOrderedSet
                s = InstructionNameOrderedSet()
                for nm in _after_off:
                    s.add(nm)
                if after_folds:
                    for nm in _after_folds:
                        s.add(nm)
                mm.ins.add_nosync_dependencies_from(s)
        for i, dy in enumerate((-1, 0, 1)):
            nc.tensor.matmul(
                ps_m,
                lhsT=WMOD1[0:64, 1 + dy:2 + dy],
                rhs=XM2[0:64, 3 * c2:3 * c2 + 3, 1 + dy:2 + dy, 2:98],
                start=False, stop=(i == 2), skip_group_check=True)
        nc.scalar.activation(
            ap(MODV, 99 * 3 * c2, [MODV[:].ap[0], [99, 3], [1, 96]]),
            ps_m, Act.Sigmoid, bias=BMOD, scale=1.0)

    def mod_rows_phi1():
        ps_m2 = psB.tile([1, NSTRIP, 3], f32, tag="ps_m")
        for i, dy in enumerate((-1, 0, 1)):
            nc.tensor.matmul(
                ps_m2,
                lhsT=WMOD2[:, 1 + dy:2 + dy],
                rhs=XM2[:, :, 2 + dy:3 + dy, 0:3],
                start=(i == 0), stop=False, skip_group_check=True)
        for i, dy in enumerate((-1, 0, 1)):
            nc.tensor.matmul(
                ps_m2,
                lhsT=WMOD1[0:64, 1 + dy:2 + dy],
                rhs=XM2[0:64, :, 2 + dy:3 + dy, 2:5],
                start=False, stop=(i == 2), skip_group_check=True)
        nc.scalar.activation(
            ap(MODV, 96, [MODV[:].ap[0], [99, NSTRIP], [1, 3]]),
            ps_m2, Act.Sigmoid, bias=BMOD, scale=1.0)

    # ---- coordinate/idx chain (DVE, shortest path to the gather);
    # high_priority so the tile scheduler keeps it ahead of the mod conv
    OCT = work.tile([NPIX, 18], f32)
    nc.gpsimd.memset(OCT, 0.0)
    with tc.high_priority():
        nc.vector.tensor_copy(OCT[0:66, :], ps_o)
        I = work.tile([NPIX, 18], f32)
        nc.vector.tensor_tensor(I, OCT, BGI, Alu.add)
        J = work.tile([NPIX, 18], f32)
        nc.vector.tensor_scalar(J, I, 12582912.0, None, Alu.add)
        I0 = work.tile([NPIX, 18], f32)
        nc.vector.tensor_scalar(I0, J, -12582912.0, None, Alu.add)
        CCL = work.tile([NPIX, 18], f32)
        nc.vector.tensor_scalar(CCL, I0, -1.0, 96.0, Alu.max, Alu.min)
        QIDX = work.tile([NPIX, ND], f32)
        nc.vector.scalar_tensor_tensor(QIDX, CCL[:, 9:18], 98.0, CCL[:, 0:9],
                                       Alu.mult, Alu.add)

        # wrapped-16 idx build on PE (bias 99 + 8 fold matmuls)
        ps_i = psA.tile([128, 72], f32, tag="psA")
        nc.tensor.matmul(ps_i, lhsT=ONES, rhs=C99, start=True, stop=False,
                         skip_group_check=True)
        for h in range(4):
            for u in range(2):
                qq = 2 * h + u
                fm = nc.tensor.matmul(
                    ps_i[:, 9 * qq:9 * qq + 9],
                    lhsT=REPLAB[32 * h:32 * h + 32, 128 * u:128 * u + 128],
                    rhs=QIDX[32 * h:32 * h + 32, :],
                    start=False,
                    stop=(h == 3 and u == 1),
                    skip_group_check=True,
                    tile_position=(32 * h, 0),
                )
                if h == 3 and u == 1:
                    _after_folds.append(fm.ins.name)
        # column reorder (q, d) -> (8d + q) during the int16 convert
        IDXC = work.tile([128, 72], dt.int16)
        nc.vector.tensor_copy(
            ap(IDXC, 0, [IDXC[:].ap[0], [1, 8], [8, ND]]),
            ap(ps_i, 0, [ps_i[:].ap[0], [ND, 8], [1, ND]]))

    # ---- mod conv phi0 strips 0-2 (PE; runs during the gather window)
    mod_rows(0)

    # ---- gathers (d 0-3, then d 4-8), d-major
    xhp = ins["xhp"]
    xhp_src = bass.AP(tensor=xhp.tensor, offset=xhp.offset,
                      ap=[[256, XHROWS], [1, 256]])
    VV = work.tile([128, ND, 256], bf)
    with tc.high_priority():
        for g0, g1 in ((0, 3), (3, 6), (6, 9)):
            nc.gpsimd.dma_gather(
                out_ap=VV[:, g0:g1, :], in_ap=xhp_src,
                idxs_ap=IDXC[:, 8 * g0:8 * g1],
                num_idxs=128 * (g1 - g0), num_idxs_reg=128 * (g1 - g0),
                elem_size=256, elem_step=256, single_packet=False)

    # ---- remaining weight math (DVE; off the gather's critical path)
    FRP = work.tile([NPIX, 18], f32)   # true frac - 0.5
    nc.vector.tensor_sub(FRP, I, I0)
    C1 = work.tile([NPIX, 18], f32)
    nc.vector.tensor_scalar(C1, I0, -1.0, None, Alu.is_ge)
    A1 = work.tile([NPIX, 18], f32)   # (frac) * C1
    nc.vector.scalar_tensor_tensor(A1, FRP, 0.5, C1, Alu.add, Alu.mult)
    W0 = work.tile([NPIX, 18], f32)   # 1 - frac = 0.5 - FRP
    nc.vector.tensor_scalar(W0, FRP, -1.0, 0.5, Alu.mult, Alu.add)
    W4 = work.tile([NPIX, 4, ND], f32)
    nc.vector.tensor_mul(W4[:, 0, :], W0[:, 9:18], W0[:, 0:9])
    nc.vector.tensor_mul(W4[:, 1, :], W0[:, 9:18], A1[:, 0:9])
    nc.vector.tensor_mul(W4[:, 2, :], A1[:, 9:18], W0[:, 0:9])
    nc.vector.tensor_mul(W4[:, 3, :], A1[:, 9:18], A1[:, 0:9])

    # ---- mod conv rest (PE) + run->(pix,d) reshape DMA
    mod_rows(1, after_folds=True)
    mod_rows_phi1()
    nc.scalar.dma_start(
        out=MODT[0:66, 0:9],
        in_=ap(MODV, 0, [MODV[:].ap[0], [9, 66], [1, 9]]))

    # ---- bilinear combine + modulation: 5-op chains per d,
    # d0-5 on DVE, d6-8 on gpsimd; transpose on PE; feat fill ACT/DVE
    S = work.tile([128, ND, C], bf)

    # modulation distributed over the 4 corner weights (saves one op/chain)
    W4M = work.tile([NPIX, 4, ND], f32)
    for j in range(4):
        nc.vector.tensor_tensor(W4M[:, j, :], W4[:, j, :], MODT[:, 0:9],
                                Alu.mult)

    def combine(eng, d):
        eng.tensor_scalar(S[:, d, :], VV[:, d, 0:64],
                          W4M[:, 0, d:d + 1], None, Alu.mult)
        eng.scalar_tensor_tensor(S[:, d, :], VV[:, d, 64:128],
                                 W4M[:, 1, d:d + 1], S[:, d, :],
                                 Alu.mult, Alu.add)
        eng.scalar_tensor_tensor(S[:, d, :], VV[:, d, 128:192],
                                 W4M[:, 2, d:d + 1], S[:, d, :],
                                 Alu.mult, Alu.add)
        eng.scalar_tensor_tensor(S[:, d, :], VV[:, d, 192:256],
                                 W4M[:, 3, d:d + 1], S[:, d, :],
                                 Alu.mult, Alu.add)

    for d in range(9):
        combine(nc.vector, d)

    for d in range(9):
        ps_f = psC.tile([C, 128], bf, tag="ps_f")
        nc.tensor.transpose(ps_f, S[:, d, :], IDENT16)
        j1 = 11 if d < 6 else 10
        nc.scalar.copy(
            ap(FP, 1 + d, [FP[:].ap[0], [196, NSTRIP], [9, j1]]),
            ap(ps_f, 0, [ps_f[:].ap[0], [NJ, NSTRIP], [1, j1]]))
        if d >= 6:
            nc.scalar.copy(
                ap(FP, 98 + 1 + d - 6, [FP[:].ap[0], [196, NSTRIP]]),
                ap(ps_f, 10, [ps_f[:].ap[0], [NJ, NSTRIP]]))

    # ---- final conv strips
    for s in range(NSTRIP):
        ps_c = psD.tile([C, 4, 96], f32, tag="ps_c")
        nc.tensor.matmul(ps_c, lhsT=WCNV[:, 0, :], rhs=ZB,
                         start=True, stop=False, skip_group_check=True)
        for t in range(9):
            dy, dx = t // 3 - 1, t % 3 - 1
            nc.tensor.matmul(
                ps_c[:, 1 - dy:3 - dy, :],
                lhsT=WCNV[:, t, :],
                rhs=FP[:, s, :, 1 + dx:97 + dx],
                start=False,
                stop=(t == 8),
                skip_group_check=True,
            )
        OUTS = loop_sb.tile([C, 4, 96], f32, tag="outs")
        nc.vector.tensor_copy(OUTS, ps_c)
        nc.sync.dma_start(out=strips_out[:, s], in_=OUTS)

    if KDEBUG:
        nc.sync.dma_start(out=outs["dbg_qidx"], in_=QIDX)
        nc.sync.dma_start(out=outs["dbg_oct"], in_=OCT)
        nc.sync.dma_start(out=outs["dbg_i"], in_=I)
        nc.sync.dma_start(out=outs["dbg_idxc"], in_=IDXC)
        nc.sync.dma_start(out=outs["dbg_w4"], in_=W4)
        nc.sync.dma_start(out=outs["dbg_modt"], in_=MODT)
        nc.sync.dma_start(out=outs["dbg_s"], in_=S)
        nc.sync.dma_start(out=outs["dbg_vv"], in_=VV)
        nc.sync.dma_start(out=outs["dbg_fp"], in_=FP)
        nc.sync.dma_start(out=outs["dbg_modv"], in_=MODV)

    ctx.close()


@functools.lru_cache(maxsize=1)
def _build_program():
    from contextlib import ExitStack

    import concourse.bacc as bacc
    import concourse.tile as tile
    from concourse import mybir

    dt = mybir.dt
    nc = bacc.Bacc("TRN2", target_bir_lowering=False, debug=False)
    ins = {
        "xhp": nc.dram_tensor("xhp", [XHROWS, 4 * C], dt.bfloat16,
                              kind="ExternalInput").ap(),
        "blobw": nc.dram_tensor("blobw", [128, BW_COLS], dt.bfloat16,
                                kind="ExternalInput").ap(),
        "blobb1": nc.dram_tensor("blobb1", [128, BB1_COLS], dt.float32,
                                 kind="ExternalInput").ap(),
        "blobb2": nc.dram_tensor("blobb2", [128, BB_COLS - BB1_COLS],
                                 dt.float32, kind="ExternalInput").ap(),
        "blob128": nc.dram_tensor("blob128", [128, 6], dt.bfloat16,
                                  kind="ExternalInput").ap(),
        "xm2a": nc.dram_tensor("xm2a", [128, 3 * 4 * 98], dt.bfloat16,
                               kind="ExternalInput").ap(),
        "xm2b": nc.dram_tensor("xm2b", [128, 3 * 4 * 98], dt.bfloat16,
                               kind="ExternalInput").ap(),
    }
    outs = {
        "strips_out": nc.dram_tensor("strips_out", [C, NSTRIP, 4, 96],
                                     dt.float32, kind="ExternalOutput").ap(),
    }
    if KDEBUG:
        outs["dbg_qidx"] = nc.dram_tensor("dbg_qidx", [NPIX, ND], dt.float32,
                                          kind="ExternalOutput").ap()
        outs["dbg_oct"] = nc.dram_tensor("dbg_oct", [NPIX, 18], dt.float32,
                                         kind="ExternalOutput").ap()
        outs["dbg_i"] = nc.dram_tensor("dbg_i", [NPIX, 18], dt.float32,
                                       kind="ExternalOutput").ap()
        outs["dbg_idxc"] = nc.dram_tensor("dbg_idxc", [128, 72], dt.int16,
                                          kind="ExternalOutput").ap()
        outs["dbg_w4"] = nc.dram_tensor("dbg_w4", [NPIX, 4, ND], dt.float32,
                                        kind="ExternalOutput").ap()
        outs["dbg_modt"] = nc.dram_tensor("dbg_modt", [128, ND], dt.float32,
                                          kind="ExternalOutput").ap()
        outs["dbg_s"] = nc.dram_tensor("dbg_s", [NPIX, ND, C], dt.bfloat16,
                                       kind="ExternalOutput").ap()
        outs["dbg_vv"] = nc.dram_tensor("dbg_vv", [128, ND, 256], dt.bfloat16,
                                        kind="ExternalOutput").ap()
        outs["dbg_fp"] = nc.dram_tensor("dbg_fp", [C, NSTRIP, 2, 98],
                                        dt.bfloat16,
                                        kind="ExternalOutput").ap()
        outs["dbg_modv"] = nc.dram_tensor("dbg_modv", [1, NSTRIP, 99],
                                          dt.float32,
                                          kind="ExternalOutput").ap()
    with ExitStack() as ctx:
        tc = ctx.enter_context(tile.TileContext(nc))
        emit_kernel(tc, outs, ins)
    nc.compile()
    return nc


def _host_inputs(inputs):
    import ml_dtypes
    bf16 = ml_dtypes.bfloat16
    arrs = {k: np.asarray(v, np.float32) for k, v in inputs.items()}
    xhps = [_make_xhp(arrs["x"][b], bf16) for b in range(4)]
    in_maps = []
    for core in range(8):
        b, part = core // 2, core % 2
        in_maps.append(_make_core_inputs(
            arrs["x"], arrs["w_off1"], arrs["b_off1"], arrs["w_off2"],
            arrs["b_off2"], arrs["w_mod"], arrs["b_mod"],
            arrs["conv_weight"], float(arrs["alpha"][0]), b, part, xhps[b]))
    return in_maps


def _assemble(results):
    out = np.zeros((4, C, H, W), np.float32)
    for core, res in enumerate(results):
        b, part = core // 2, core % 2
        i0 = 6 * part
        strips = res["strips_out"]
        for s in range(NSTRIP):
            r0 = 9 * (i0 + s) - 1
            if r0 < 0:
                out[b][:, 0:r0 + 4, :] = strips[:, s, -r0:, :]
            elif r0 + 4 <= H:
                out[b][:, r0:r0 + 4, :] = strips[:, s]
    return out


def kernel(**inputs) -> np.ndarray:
    from concourse.bass_utils import run_bass_kernel_spmd

    nc = _build_program()
    in_maps = _host_inputs(inputs)
    res = run_bass_kernel_spmd(nc, in_maps, core_ids=list(range(8)))
    return _assemble(res.results)


if __name__ == "__main__":
    d = dict(np.load("/root/problem/inputs_cache.npz"))
    out = kernel(**d)
    ref = np.load("/root/problem/expected_np.npy")
    err = np.abs(out - ref).max()
    print("absmax err:", err, "rel:", err / np.abs(ref).max())
